# revision 1
# baseline (speedup 1.0000x reference)
"""CARE-GNN Trainium2 kernel (nn_CAREGNN_62199716381202).

Strategy (graph/data parallel, 8 NeuronCores):
- Shard destination nodes across the 8 cores (6250 dsts each); each core owns
  the edges incident (by dst) to its shard.
- Segment-mean becomes, per core: dst-sorted edge gather (dma_gather) of
  source-node features + one-hot scatter matmul accumulated in PSUM per
  128-dst block.
- fp32 exactness: features stored as bf16 (hi, lo) pairs; hi+lo == fp32 value
  to ~1e-7 relative.  Gathers move bf16 [hi|lo] rows; the scatter matmul
  accumulates both planes in fp32 PSUM; planes are summed afterwards.
- int16 gather indices can only address 32768 rows, so the node table is
  split at row 32768 into two gather source views (low/high); each 128-dst
  block's edges are split accordingly and both halves accumulate into the
  same PSUM block.
- Small Linear weights are replicated; h (layer-0 output) is AllGathered
  (bf16 hi/lo rows) between layers, in chunks overlapped with layer-0
  epilogue work.
"""

import sys

if "/opt/trn_rl_repo" not in sys.path:
    sys.path.insert(0, "/opt/trn_rl_repo")

import numpy as np
import ml_dtypes

BF16 = ml_dtypes.bfloat16

import concourse.bass as bass
import concourse.bacc as bacc
import concourse.mybir as mybir
import concourse.tile as tile
from concourse.bass_utils import run_bass_kernel_spmd

F32 = mybir.dt.float32
BF = mybir.dt.bfloat16
I16 = mybir.dt.int16
ADD = mybir.AluOpType.add
SUB = mybir.AluOpType.subtract
MULT = mybir.AluOpType.mult
ISEQ = mybir.AluOpType.is_equal
TANH = mybir.ActivationFunctionType.Tanh


class Cfg:
    def __init__(self, N=50000, E=500000, n_cores=8, split=32768, win=2,
                 n_ag=1):
        self.N = N          # nodes
        self.E = E          # edges per etype
        self.D = 128        # input feature dim (must be 128)
        self.HID = 64       # hidden dim (must be 64)
        self.C = 2          # classes
        self.NET = 3        # edge types
        self.n_cores = n_cores
        self.split = split  # int16-index split point (<= 32768)
        self.win = win      # dst blocks per processing window
        self.n_ag = n_ag    # AllGather chunks
        assert N % n_cores == 0
        self.ND = N // n_cores            # dsts per core
        self.NB = (self.ND + 127) // 128  # 128-dst blocks per core
        self.windows = [
            (s, min(win, self.NB - s)) for s in range(0, self.NB, win)
        ]

    def bs(self, b):  # dsts in block b
        return min(128, self.ND - b * 128)


def _wrap16(flat):
    """idx list position j -> [j%16, j//16] int16 layout, replicated to all
    8 Q7-core partition groups (tx core reads its own group)."""
    assert flat.size % 16 == 0
    w = np.ascontiguousarray(flat.reshape(-1, 16).T).astype(np.int16)
    return np.tile(w, (8, 1))


def host_prep(cfg, inputs):
    """Build per-core input maps.  Returns (in_maps, CA, CB)."""
    feat = np.asarray(inputs["feat"], np.float32)
    srcs = [np.asarray(inputs[f"src{i}"]) for i in range(cfg.NET)]
    dsts = [np.asarray(inputs[f"dst{i}"]) for i in range(cfg.NET)]

    # hi/lo bf16 split of node features: row = [hi(128) | lo(128)]
    hi = feat.astype(BF16)
    lo = (feat - hi.astype(np.float32)).astype(BF16)
    X = np.concatenate([hi, lo], axis=1)  # [N, 256] bf16

    # pass 1: per (core, etype, block, half) edge counts -> CA, CB
    percore = []
    CA = CB = 1
    for k in range(cfg.n_cores):
        rows = []
        for i in range(cfg.NET):
            sel = (dsts[i] >= k * cfg.ND) & (dsts[i] < (k + 1) * cfg.ND)
            dl = (dsts[i][sel] - k * cfg.ND).astype(np.int64)
            s = srcs[i][sel].astype(np.int64)
            o = np.argsort(dl, kind="stable")
            dl, s = dl[o], s[o]
            b = dl >> 7
            half = (s >= cfg.split).astype(np.int64)
            grp = b * 2 + half
            o2 = np.argsort(grp, kind="stable")
            dl, s, b, half, grp = dl[o2], s[o2], b[o2], half[o2], grp[o2]
            if len(grp):
                newg = np.r_[True, grp[1:] != grp[:-1]]
                starts = np.flatnonzero(newg)
                lens = np.diff(np.r_[starts, len(grp)])
                cum = np.arange(len(grp)) - np.repeat(starts, lens)
                nA = np.max(np.where(half == 0, cum, -1)) + 1 if (half == 0).any() else 0
                nB = np.max(np.where(half == 1, cum, -1)) + 1 if (half == 1).any() else 0
            else:
                cum = np.zeros(0, np.int64)
                nA = nB = 0
            CA = max(CA, -(-int(nA) // 128))
            CB = max(CB, -(-int(nB) // 128))
            rows.append((dl, s, b, half, cum))
        percore.append(rows)

    LA = cfg.NET * cfg.NB * CA * 128
    LB = cfg.NET * cfg.NB * CB * 128
    GA = cfg.NET * cfg.NB * CA
    GB = cfg.NET * cfg.NB * CB

    # shared (replicated) small tensors
    Wm = np.asarray(inputs["Wm"], np.float32)
    bm = np.asarray(inputs["bm"], np.float32).reshape(cfg.C, 1)
    W0 = np.asarray(inputs["W0"], np.float32)
    b0 = np.asarray(inputs["b0"], np.float32).reshape(cfg.HID, 1)
    W1 = np.asarray(inputs["W1"], np.float32)
    b1 = np.asarray(inputs["b1"], np.float32).reshape(cfg.C, 1)
    p0 = np.tile(np.asarray(inputs["p0"], np.float32), (128, 1))
    p1 = np.tile(np.asarray(inputs["p1"], np.float32), (128, 1))
    gmax = cfg.win * max(CA, CB)
    iota = np.ascontiguousarray(
        np.broadcast_to(
            np.arange(128, dtype=np.float32).astype(BF16)[None, :, None],
            (128, 128, gmax),
        ).reshape(128, 128 * gmax)
    )
    ident = np.eye(128, dtype=np.float32)

    in_maps = []
    for k in range(cfg.n_cores):
        idxA = np.zeros(LA, np.int64)
        dlA = np.full(LA, -1.0, np.float32)
        idxB = np.zeros(LB, np.int64)
        dlB = np.full(LB, -1.0, np.float32)
        icnt = np.ones((cfg.NET, cfg.NB * 128), np.float32)
        for i in range(cfg.NET):
            dl, s, b, half, cum = percore[k][i]
            cnt = np.bincount(dl, minlength=cfg.ND)
            icnt[i, : cfg.ND] = 1.0 / np.maximum(cnt, 1.0)
            mA = half == 0
            mB = half == 1
            posA = (i * cfg.NB + b[mA]) * CA * 128 + cum[mA]
            idxA[posA] = s[mA]
            dlA[posA] = dl[mA] - b[mA] * 128
            posB = (i * cfg.NB + b[mB]) * CB * 128 + cum[mB]
            idxB[posB] = s[mB] - cfg.split
            dlB[posB] = dl[mB] - b[mB] * 128
        x_own = np.zeros((cfg.NB * 128, 256), BF16)
        x_own[: cfg.ND] = X[k * cfg.ND : (k + 1) * cfg.ND]
        in_maps.append(
            {
                "x_hilo": X,
                "x_own": x_own,
                "idxA": _wrap16(idxA),
                "idxB": _wrap16(idxB),
                "dstA": np.ascontiguousarray(
                    dlA.reshape(GA, 128).T
                ).astype(BF16),
                "dstB": np.ascontiguousarray(
                    dlB.reshape(GB, 128).T
                ).astype(BF16),
                "icnt": np.ascontiguousarray(
                    icnt.reshape(cfg.NET * cfg.NB, 128).T
                ),
                "Wm": Wm, "bm": bm, "W0": W0, "b0": b0, "W1": W1, "b1": b1,
                "p0": p0, "p1": p1, "iota": iota, "ident": ident,
            }
        )
    return in_maps, CA, CB


def build_nc(cfg, CA, CB, debug=False, reps=1, ablate=()):
    N, ND, NB, NET, HID, C = cfg.N, cfg.ND, cfg.NB, cfg.NET, cfg.HID, cfg.C
    SPLIT = cfg.split
    WIN = cfg.win
    LA = NET * NB * CA * 128
    LB = NET * NB * CB * 128
    GA = NET * NB * CA
    GB = NET * NB * CB
    GMAX = WIN * max(CA, CB)

    nc = bacc.Bacc(trn_type="TRN2", num_devices=cfg.n_cores,
                   num_swdge_queues=4)

    x_hilo = nc.dram_tensor("x_hilo", [N, 256], BF, kind="ExternalInput")
    x_own = nc.dram_tensor("x_own", [NB * 128, 256], BF, kind="ExternalInput")
    idxA_d = nc.dram_tensor("idxA", [128, LA // 16], I16, kind="ExternalInput")
    idxB_d = nc.dram_tensor("idxB", [128, LB // 16], I16, kind="ExternalInput")
    dstA_d = nc.dram_tensor("dstA", [128, GA], BF, kind="ExternalInput")
    dstB_d = nc.dram_tensor("dstB", [128, GB], BF, kind="ExternalInput")
    icnt_d = nc.dram_tensor("icnt", [128, NET * NB], F32, kind="ExternalInput")
    Wm_d = nc.dram_tensor("Wm", [128, C], F32, kind="ExternalInput")
    bm_d = nc.dram_tensor("bm", [C, 1], F32, kind="ExternalInput")
    W0_d = nc.dram_tensor("W0", [128, HID], F32, kind="ExternalInput")
    b0_d = nc.dram_tensor("b0", [HID, 1], F32, kind="ExternalInput")
    W1_d = nc.dram_tensor("W1", [HID, C], F32, kind="ExternalInput")
    b1_d = nc.dram_tensor("b1", [C, 1], F32, kind="ExternalInput")
    p0_d = nc.dram_tensor("p0", [128, NET], F32, kind="ExternalInput")
    p1_d = nc.dram_tensor("p1", [128, NET], F32, kind="ExternalInput")
    iota_d = nc.dram_tensor("iota", [128, 128 * GMAX], BF,
                            kind="ExternalInput")
    ident_d = nc.dram_tensor("ident", [128, 128], F32, kind="ExternalInput")
    outT_d = nc.dram_tensor("outT", [C, ND], F32, kind="ExternalOutput")
    simT_d = nc.dram_tensor("simT", [C, ND], F32, kind="ExternalOutput")

    # AllGather chunk boundaries: after these window indices, gather the
    # rows finished so far.
    nw = len(cfg.windows)
    ag_after = set()
    for c in range(1, cfg.n_ag + 1):
        ag_after.add(min(nw - 1, (c * nw) // cfg.n_ag - 1))

    with tile.TileContext(nc) as tc:
        with (
            tc.tile_pool(name="const", bufs=1) as cp,
            tc.tile_pool(name="big", bufs=1) as bigp,
            tc.tile_pool(name="gath", bufs=2) as gp,
            tc.tile_pool(name="sgen", bufs=2) as sp,
            tc.tile_pool(name="work", bufs=2) as wp,
            tc.tile_pool(name="ps", bufs=2, space="PSUM") as pp,
            tc.tile_pool(name="pt", bufs=2, space="PSUM") as ptp,
            tc.tile_pool(name="po", bufs=2, space="PSUM") as pop,
            tc.tile_pool(name="dram", bufs=1, space="DRAM") as dp,
        ):
            # ---- resident constants / streams ----
            idxA = cp.tile([128, LA // 16], I16)
            idxB = cp.tile([128, LB // 16], I16)
            nc.sync.dma_start(out=idxA[:, :], in_=idxA_d[:, :])
            nc.sync.dma_start(out=idxB[:, :], in_=idxB_d[:, :])
            dstA = cp.tile([128, GA], BF)
            dstB = cp.tile([128, GB], BF)
            nc.sync.dma_start(out=dstA[:], in_=dstA_d[:, :])
            nc.sync.dma_start(out=dstB[:], in_=dstB_d[:, :])
            icnt = cp.tile([128, NET * NB], F32)
            nc.sync.dma_start(out=icnt[:], in_=icnt_d[:, :])
            Wm_s = cp.tile([128, C], F32)
            bm_s = cp.tile([C, 1], F32)
            W0_s = cp.tile([128, HID], F32)
            b0_s = cp.tile([HID, 1], F32)
            W1_s = cp.tile([HID, C], F32)
            b1_s = cp.tile([C, 1], F32)
            p0_s = cp.tile([128, NET], F32)
            p1_s = cp.tile([128, NET], F32)
            iota_s = cp.tile([128, 128, GMAX], BF)
            ident_s = cp.tile([128, 128], F32)
            for t_, d_ in [
                (Wm_s, Wm_d), (bm_s, bm_d), (W0_s, W0_d), (b0_s, b0_d),
                (W1_s, W1_d), (b1_s, b1_d), (p0_s, p0_d), (p1_s, p1_d),
                (iota_s, iota_d.rearrange("p (j g) -> p j g", j=128)),
                (ident_s, ident_d),
            ]:
                nc.sync.dma_start(out=t_[:], in_=d_[:, :])

            hacc = bigp.tile([128, NB, 128], F32)   # layer-0 weighted agg
            h1acc = bigp.tile([128, NB, HID], F32)  # layer-1 weighted agg
            hnat = bigp.tile([128, NB, HID], F32)   # layer-0 output (natural)

            h_loc = dp.tile([ND, 128], BF)
            h_ag = dp.tile([N, 128], BF)
            h_ag_v = h_ag[:, :].rearrange("(k r) d -> k r d", k=cfg.n_cores)

            qctr = [0]

            def gather_batched(gtile, src, idx_tile, gc0, nchunks, elem):
                """dma_gather in sub-calls of <=8 chunks (1024 idx hard
                limit), round-robin over the 4 SWDGE queues."""
                c = 0
                while c < nchunks:
                    cc = min(8, nchunks - c)
                    n = cc * 128
                    q = qctr[0] % 4
                    qctr[0] += 1
                    nc.gpsimd.dma_gather(
                        gtile[:, c : c + cc, :], src,
                        idx_tile[:, (gc0 + c) * 8 : (gc0 + c + cc) * 8],
                        n, n, elem, queue_num=q,
                    )
                    c += cc

            def agg_window(layer, src_lo, src_hi, elem, acc, p_s, b0w, wb):
                """Aggregate all etypes for one window of dst blocks."""
                ecols = elem // 2
                for i in range(NET):
                    gc0A = (i * NB + b0w) * CA
                    gc0B = (i * NB + b0w) * CB
                    gA = gp.tile([128, WIN * CA, elem], BF, tag=f"gA{layer}")
                    gB = gp.tile([128, WIN * CB, elem], BF, tag=f"gB{layer}")
                    if "gather" in ablate:
                        nc.vector.memset(gA[:, : wb * CA, :], 0)
                        nc.vector.memset(gB[:, : wb * CB, :], 0)
                    else:
                        gather_batched(gA, src_lo, idxA, gc0A, wb * CA, elem)
                        gather_batched(gB, src_hi, idxB, gc0B, wb * CB, elem)
                    SA = sp.tile([128, 128, WIN * CA], BF, tag="SA")
                    SB = sp.tile([128, 128, WIN * CB], BF, tag="SB")
                    if "sgen" in ablate:
                        nc.vector.memset(SA[:, :, : wb * CA], 0)
                        nc.vector.memset(SB[:, :, : wb * CB], 0)
                    else:
                        nc.vector.tensor_tensor(
                            SA[:, :, : wb * CA],
                            iota_s[:, :, : wb * CA],
                            dstA[:, gc0A : gc0A + wb * CA].unsqueeze(
                                1
                            ).broadcast_to([128, 128, wb * CA]),
                            ISEQ,
                        )
                        nc.vector.tensor_tensor(
                            SB[:, :, : wb * CB],
                        iota_s[:, :, : wb * CB],
                            dstB[:, gc0B : gc0B + wb * CB].unsqueeze(
                                1
                            ).broadcast_to([128, 128, wb * CB]),
                            ISEQ,
                        )
                    ps = pp.tile([128, WIN, 2 * ecols], F32, tag="ps")
                    for bb in range(wb):
                        if "mm" in ablate:
                            nc.tensor.matmul(
                                ps[:, bb, :], SA[:, :, bb * CA],
                                gA[:, bb * CA, :], start=True, stop=True,
                            )
                            continue
                        for c in range(CA):
                            nc.tensor.matmul(
                                ps[:, bb, :],
                                SA[:, :, bb * CA + c],
                                gA[:, bb * CA + c, :],
                                start=(c == 0),
                                stop=False,
                            )
                        for c in range(CB):
                            nc.tensor.matmul(
                                ps[:, bb, :],
                                SB[:, :, bb * CB + c],
                                gB[:, bb * CB + c, :],
                                start=False,
                                stop=(c == CB - 1),
                            )
                    t = wp.tile([128, WIN, ecols], F32, tag=f"t{layer}")
                    # hw: only one non-scalar input may live in PSUM
                    nc.scalar.copy(t[:, :wb, :], ps[:, :wb, 0:ecols])
                    nc.vector.tensor_tensor(
                        t[:, :wb, :], t[:, :wb, :],
                        ps[:, :wb, ecols : 2 * ecols], ADD,
                    )
                    hr = wp.tile([128, WIN, ecols], F32, tag=f"hr{layer}")
                    for bb in range(wb):
                        nc.scalar.activation(
                            hr[:, bb, :], t[:, bb, :], TANH,
                            scale=icnt[:, i * NB + b0w + bb
                                       : i * NB + b0w + bb + 1],
                        )
                    accs = acc[:, b0w : b0w + wb, :]
                    if i == 0:
                        nc.scalar.mul(accs, hr[:, :wb, :], p_s[:, 0:1])
                    else:
                        tmp = wp.tile([128, WIN, ecols], F32, tag=f"tm{layer}")
                        nc.scalar.mul(
                            tmp[:, :wb, :], hr[:, :wb, :], p_s[:, i : i + 1]
                        )
                        nc.vector.tensor_tensor(
                            accs, accs, tmp[:, :wb, :], ADD
                        )

            for _rep in range(reps):
                # ================= LAYER 0 =================
                ag_row0 = [0]
                for wi, (b0w, wb) in enumerate(cfg.windows):
                    agg_window(0, x_hilo[0:SPLIT, :], x_hilo[SPLIT:N, :],
                               256, hacc, p0_s, b0w, wb)
                    # epilogue: residual + W0 + sim + h rows
                    fx = gp.tile([128, WIN, 256], BF, tag="fx")
                    nc.sync.dma_start(
                        out=fx[:, :wb, :],
                        in_=x_own[b0w * 128 : (b0w + wb) * 128, :].rearrange(
                            "(b p) d -> p b d", p=128
                        ),
                    )
                    ff = wp.tile([128, WIN, 128], F32, tag="ff")
                    nc.vector.tensor_tensor(
                        ff[:, :wb, :], fx[:, :wb, 0:128],
                        fx[:, :wb, 128:256], ADD,
                    )
                    h0 = wp.tile([128, WIN, 128], F32, tag="h0")
                    nc.vector.tensor_tensor(
                        h0[:, :wb, :], hacc[:, b0w : b0w + wb, :],
                        ff[:, :wb, :], ADD,
                    )
                    nc.scalar.activation(h0[:, :wb, :], h0[:, :wb, :], TANH)
                    so = wp.tile([C, WIN * 128], F32, tag="so")
                    for bb in range(wb):
                        b = b0w + bb
                        n = cfg.bs(b)
                        ptt = ptp.tile([128, 128], F32, tag="ptt")
                        nc.tensor.transpose(ptt[:], h0[:, bb, :], ident_s[:])
                        hT = wp.tile([128, 128], F32, tag="hT")
                        nc.vector.tensor_copy(hT[:], ptt[:])
                        po = pop.tile([HID, 128], F32, tag="po")
                        nc.tensor.matmul(po[:], W0_s[:], hT[:])
                        o_sb = wp.tile([HID, 128], F32, tag="osb")
                        nc.vector.tensor_scalar(
                            o_sb[:], po[:], b0_s[:, 0:1], None, ADD
                        )
                        pt2 = ptp.tile([128, HID], F32, tag="ptt")
                        nc.tensor.transpose(
                            pt2[:], o_sb[:], ident_s[0:HID, 0:HID]
                        )
                        nc.vector.tensor_copy(hnat[:, b, :], pt2[:])
                        hl = wp.tile([128, 128], BF, tag="hl")
                        nc.vector.tensor_copy(hl[:, 0:HID], pt2[:])
                        t32 = wp.tile([128, HID], F32, tag="t32")
                        nc.vector.tensor_copy(t32[:], hl[:, 0:HID])
                        nc.vector.tensor_tensor(
                            hl[:, HID : 2 * HID], pt2[:], t32[:], SUB
                        )
                        nc.sync.dma_start(
                            out=h_loc[b * 128 : b * 128 + n, :],
                            in_=hl[0:n, :],
                        )
                        # sim = tanh(feat @ Wm + bm)
                        ptf = ptp.tile([128, 128], F32, tag="ptt")
                        nc.tensor.transpose(ptf[:], ff[:, bb, :], ident_s[:])
                        fT = wp.tile([128, 128], F32, tag="fT")
                        nc.vector.tensor_copy(fT[:], ptf[:])
                        psim = pop.tile([C, 128], F32, tag="po")
                        nc.tensor.matmul(psim[:], Wm_s[:], fT[:])
                        nc.scalar.activation(
                            so[:, bb * 128 : (bb + 1) * 128], psim[:], TANH,
                            bias=bm_s[:, 0:1],
                        )
                    wcols = min(wb * 128, ND - b0w * 128)
                    nc.sync.dma_start(
                        out=simT_d[:, b0w * 128 : b0w * 128 + wcols],
                        in_=so[:, 0:wcols],
                    )
                    # chunked AllGather of finished h rows
                    if wi in ag_after:
                        r0 = ag_row0[0]
                        r1 = min(ND, (b0w + wb) * 128)
                        if r1 > r0 and "ag" not in ablate:
                            nc.gpsimd.collective_compute(
                                "AllGather",
                                mybir.AluOpType.bypass,
                                replica_groups=[list(range(cfg.n_cores))],
                                ins=[h_loc[r0:r1, :].opt()],
                                outs=[h_ag_v[:, r0:r1, :].opt()],
                            )
                        ag_row0[0] = r1

                # ================= LAYER 1 =================
                for (b0w, wb) in cfg.windows:
                    agg_window(1, h_ag[0:SPLIT, :], h_ag[SPLIT:N, :],
                               128, h1acc, p1_s, b0w, wb)
                    h2 = wp.tile([128, WIN, HID], F32, tag="h2")
                    nc.vector.tensor_tensor(
                        h2[:, :wb, :], h1acc[:, b0w : b0w + wb, :],
                        hnat[:, b0w : b0w + wb, :], ADD,
                    )
                    nc.scalar.activation(h2[:, :wb, :], h2[:, :wb, :], TANH)
                    oo = wp.tile([C, WIN * 128], F32, tag="oo")
                    for bb in range(wb):
                        pt3 = ptp.tile([HID, 128], F32, tag="ptt")
                        nc.tensor.transpose(pt3[:], h2[:, bb, :], ident_s[:])
                        h2T = wp.tile([HID, 128], F32, tag="h2T")
                        nc.vector.tensor_copy(h2T[:], pt3[:])
                        po2 = pop.tile([C, 128], F32, tag="po")
                        nc.tensor.matmul(po2[:], W1_s[:], h2T[:])
                        nc.vector.tensor_scalar(
                            oo[:, bb * 128 : (bb + 1) * 128], po2[:],
                            b1_s[:, 0:1], None, ADD,
                        )
                    wcols = min(wb * 128, ND - b0w * 128)
                    nc.sync.dma_start(
                        out=outT_d[:, b0w * 128 : b0w * 128 + wcols],
                        in_=oo[:, 0:wcols],
                    )

    nc.compile()
    return nc


_CACHE = {}


def _get_nc(cfg, CA, CB):
    key = (cfg.N, cfg.E, cfg.n_cores, CA, CB)
    if key not in _CACHE:
        _CACHE[key] = build_nc(cfg, CA, CB)
    return _CACHE[key]


def kernel(**inputs):
    cfg = Cfg()
    in_maps, CA, CB = host_prep(cfg, inputs)
    nc = _get_nc(cfg, CA, CB)
    res = run_bass_kernel_spmd(nc, in_maps, core_ids=list(range(cfg.n_cores)))
    out = np.concatenate(
        [r["outT"] for r in res.results], axis=1
    ).T.astype(np.float32)
    sim = np.concatenate(
        [r["simT"] for r in res.results], axis=1
    ).T.astype(np.float32)
    return (np.ascontiguousarray(out), np.ascontiguousarray(sim))



# revision 5
# speedup vs baseline: 1.0667x; 1.0667x over previous
"""CARE-GNN Trainium2 kernel (nn_CAREGNN_62199716381202), v2.

Strategy (graph/data parallel, 8 NeuronCores):
- Shard destination nodes across the 8 cores (6250 dsts each); each core owns
  the edges incident (by dst) to its shard, sorted by dst, split into
  low-src / high-src streams (int16 gather-index limit at 32768).
- Per 128-dst block and etype, edges are gathered (dma_gather, bf16 rows,
  256B descriptors) into edge-major chunks G [128 slots, 128 feat].
- Segment-MEAN via one flipped one-hot matmul per chunk:
      psum[feat, dst] += G[slot, feat]^T @ S[slot, dst]
  where S = (iota == lbl) * (1/deg)  -- the mean scale is baked into S, so
  PSUM holds the per-etype mean directly, already in transposed [feat, dst]
  layout.  The transposed layout feeds tanh / p-weighted sum / residual /
  Linear (W @ h) with no per-block PE transposes.
- bf16 everywhere data-sized (inputs quantized ~0.4%, fine for rel<2e-2).
- Both layers share identical edge streams (same graph): one set of
  idx/label/icnt metadata, two gather sources (X rows, AllGathered H rows).
- AllGather of layer-0 H rows (bf16, 128-padded) is chunked to overlap with
  layer-0 compute.
"""

import sys

if "/opt/trn_rl_repo" not in sys.path:
    sys.path.insert(0, "/opt/trn_rl_repo")

import numpy as np
import ml_dtypes

BF16 = ml_dtypes.bfloat16

import concourse.bass as bass
import concourse.bacc as bacc
import concourse.mybir as mybir
import concourse.tile as tile
from concourse.bass_utils import run_bass_kernel_spmd

F32 = mybir.dt.float32
BF = mybir.dt.bfloat16
I16 = mybir.dt.int16
ADD = mybir.AluOpType.add
MULT = mybir.AluOpType.mult
ISEQ = mybir.AluOpType.is_equal
TANH = mybir.ActivationFunctionType.Tanh


class Cfg:
    def __init__(self, N=50000, E=500000, n_cores=8, split=32768, n_ag=4):
        self.N = N
        self.E = E
        self.D = 128
        self.HID = 64
        self.C = 2
        self.NET = 3
        self.n_cores = n_cores
        self.split = split
        self.n_ag = n_ag
        assert N % n_cores == 0
        self.ND = N // n_cores
        self.NB = (self.ND + 127) // 128

    def bs(self, b):
        return min(128, self.ND - b * 128)


def _wrap16(flat):
    w = np.ascontiguousarray(flat.reshape(-1, 16).T).astype(np.int16)
    return np.tile(w, (8, 1))


def host_prep(cfg, inputs):
    """Build per-core input maps. Returns (in_maps, CA, CB)."""
    feat = np.asarray(inputs["feat"], np.float32)
    srcs = [np.asarray(inputs[f"src{i}"]) for i in range(cfg.NET)]
    dsts = [np.asarray(inputs[f"dst{i}"]) for i in range(cfg.NET)]

    x_rows = feat.astype(BF16)                      # [N, 128] gather source
    xT = np.ascontiguousarray(feat.T)               # [128, N] f32

    # pass 1: per (core, etype, block, half) counts -> CA, CB
    percore = []
    CA = CB = 1
    for k in range(cfg.n_cores):
        rows = []
        for i in range(cfg.NET):
            sel = (dsts[i] >= k * cfg.ND) & (dsts[i] < (k + 1) * cfg.ND)
            dl = (dsts[i][sel] - k * cfg.ND).astype(np.int64)
            s = srcs[i][sel].astype(np.int64)
            o = np.argsort(dl, kind="stable")
            dl, s = dl[o], s[o]
            deg = np.bincount(dl, minlength=cfg.ND)
            b = dl >> 7
            half = (s >= cfg.split).astype(np.int64)
            for b_ in range(cfg.NB):
                mb = b == b_
                nA = int((mb & (half == 0)).sum())
                nB = int((mb & (half == 1)).sum())
                CA = max(CA, -(-nA // 128))
                CB = max(CB, -(-nB // 128))
            rows.append((dl, s, b, half, deg))
        percore.append(rows)

    # layer-1 table row remap (chunk-major AllGather layout)
    bounds = [0]
    for c in range(1, cfg.n_ag + 1):
        bb = min(cfg.NB - 1, (c * cfg.NB) // cfg.n_ag - 1)
        bounds.append(min(cfg.ND, (bb + 1) * 128))
    bounds = sorted(set(bounds))
    remap = np.zeros(cfg.N, np.int64)
    nodes = np.arange(cfg.N)
    kk, rr = nodes // cfg.ND, nodes % cfg.ND
    cum = 0
    for ci in range(len(bounds) - 1):
        r0, r1 = bounds[ci], bounds[ci + 1]
        m = (rr >= r0) & (rr < r1)
        remap[m] = cum + kk[m] * (r1 - r0) + (rr[m] - r0)
        cum += cfg.n_cores * (r1 - r0)

    # layer-1 chunk maxima (remapped halves)
    CA1 = CB1 = 1
    for k in range(cfg.n_cores):
        for i in range(cfg.NET):
            dl, s, b, half, deg = percore[k][i]
            h1 = (remap[s] >= cfg.split).astype(np.int64)
            for b_ in range(cfg.NB):
                mb = b == b_
                nA = int((mb & (h1 == 0)).sum())
                nB = int((mb & (h1 == 1)).sum())
                CA1 = max(CA1, -(-nA // 128))
                CB1 = max(CB1, -(-nB // 128))

    LA = cfg.NET * cfg.NB * CA * 128
    LB = cfg.NET * cfg.NB * CB * 128
    GA = cfg.NET * cfg.NB * CA   # lo chunks
    GB = cfg.NET * cfg.NB * CB   # hi chunks
    LA1 = cfg.NET * cfg.NB * CA1 * 128
    LB1 = cfg.NET * cfg.NB * CB1 * 128
    GA1 = cfg.NET * cfg.NB * CA1
    GB1 = cfg.NET * cfg.NB * CB1

    Wm = np.asarray(inputs["Wm"], np.float32).astype(BF16)   # [128, 2]
    bm = np.asarray(inputs["bm"], np.float32).reshape(cfg.C, 1)
    W0 = np.asarray(inputs["W0"], np.float32).astype(BF16)   # [128, 64]
    b0 = np.asarray(inputs["b0"], np.float32).reshape(cfg.HID, 1)
    W1 = np.asarray(inputs["W1"], np.float32).astype(BF16)   # [64, 2]
    b1 = np.asarray(inputs["b1"], np.float32).reshape(cfg.C, 1)
    p0 = np.tile(np.asarray(inputs["p0"], np.float32), (128, 1))
    p1 = np.tile(np.asarray(inputs["p1"], np.float32), (128, 1))
    CH = max(CA, CB, CA1, CB1)
    iota3 = np.ascontiguousarray(
        np.broadcast_to(
            np.arange(128, dtype=np.float32).astype(BF16)[None, :, None],
            (128, 128, CH),
        ).reshape(128, 128 * CH)
    )
    ident = np.eye(128, dtype=np.float32).astype(BF16)

    def build_meta(k, CXA, CXB, use_remap):
        LAx, LBx = cfg.NET * cfg.NB * CXA * 128, cfg.NET * cfg.NB * CXB * 128
        GAx, GBx = cfg.NET * cfg.NB * CXA, cfg.NET * cfg.NB * CXB
        idxA = np.zeros(LAx, np.int64)
        idxB = np.zeros(LBx, np.int64)
        lblA = np.full((128, GAx), -7.0, np.float32)
        lblB = np.full((128, GBx), -7.0, np.float32)
        icnA = np.zeros((128, GAx), np.float32)
        icnB = np.zeros((128, GBx), np.float32)
        for i in range(cfg.NET):
            dl, s, b, half, deg = percore[k][i]
            sv = remap[s] if use_remap else s
            hv = (sv >= cfg.split).astype(np.int64)
            ic = 1.0 / np.maximum(deg, 1.0)
            for half_, (idx_, lbl_, icn_, CX) in (
                (0, (idxA, lblA, icnA, CXA)),
                (1, (idxB, lblB, icnB, CXB)),
            ):
                m = hv == half_
                dlh, sh = dl[m], sv[m]
                bh = dlh >> 7
                cnt = np.bincount(bh, minlength=cfg.NB)
                start = np.zeros(cfg.NB + 1, np.int64)
                np.cumsum(cnt, out=start[1:])
                j = np.arange(len(dlh)) - start[bh]
                pos = ((i * cfg.NB + bh) * CX + (j >> 7)) * 128 + (j & 127)
                idx_[pos] = sh - (cfg.split if half_ else 0)
                ch = (i * cfg.NB + bh) * CX + (j >> 7)
                lbl_[j & 127, ch] = dlh - bh * 128
                icn_[j & 127, ch] = ic[dlh]
        return idxA, idxB, lblA, lblB, icnA, icnB

    in_maps = []
    for k in range(cfg.n_cores):
        idxA, idxB, lblA, lblB, icnA, icnB = build_meta(k, CA, CB, False)
        idxA1, idxB1, lblA1, lblB1, icnA1, icnB1 = build_meta(
            k, CA1, CB1, True)
        xo = np.zeros((128, cfg.NB * 128), BF16)
        xo[:, : cfg.ND] = xT[:, k * cfg.ND : (k + 1) * cfg.ND].astype(BF16)
        in_maps.append(
            {
                "x_rows": x_rows,
                "x_ownT": xo,
                "idxA": _wrap16(idxA),
                "idxB": _wrap16(idxB),
                "lblA": lblA.astype(BF16),
                "lblB": lblB.astype(BF16),
                "icnA": icnA.astype(BF16),
                "icnB": icnB.astype(BF16),
                "idxA1": _wrap16(idxA1),
                "idxB1": _wrap16(idxB1),
                "lblA1": lblA1.astype(BF16),
                "lblB1": lblB1.astype(BF16),
                "icnA1": icnA1.astype(BF16),
                "icnB1": icnB1.astype(BF16),
                "Wm": Wm, "bm": bm, "W0": W0, "b0": b0, "W1": W1, "b1": b1,
                "p0": p0, "p1": p1, "iota3": iota3, "ident": ident,
            }
        )
    return in_maps, (CA, CB, CA1, CB1)


def build_nc(cfg, CAB, debug=False):
    CA, CB, CA1, CB1 = CAB
    N, ND, NB, NET, HID, C = cfg.N, cfg.ND, cfg.NB, cfg.NET, cfg.HID, cfg.C
    SPLIT = cfg.split
    LA = NET * NB * CA * 128
    LB = NET * NB * CB * 128
    GA = NET * NB * CA
    GB = NET * NB * CB
    LA1 = NET * NB * CA1 * 128
    LB1 = NET * NB * CB1 * 128
    GA1 = NET * NB * CA1
    GB1 = NET * NB * CB1
    CH = max(CA, CB, CA1, CB1)

    nc = bacc.Bacc(trn_type="TRN2", num_devices=cfg.n_cores,
                   num_swdge_queues=4)

    x_rows_d = nc.dram_tensor("x_rows", [N, 128], BF, kind="ExternalInput")
    x_ownT_d = nc.dram_tensor("x_ownT", [128, NB * 128], BF, kind="ExternalInput")
    idxA_d = nc.dram_tensor("idxA", [128, LA // 16], I16, kind="ExternalInput")
    idxB_d = nc.dram_tensor("idxB", [128, LB // 16], I16, kind="ExternalInput")
    lblA_d = nc.dram_tensor("lblA", [128, GA], BF, kind="ExternalInput")
    lblB_d = nc.dram_tensor("lblB", [128, GB], BF, kind="ExternalInput")
    icnA_d = nc.dram_tensor("icnA", [128, GA], BF, kind="ExternalInput")
    icnB_d = nc.dram_tensor("icnB", [128, GB], BF, kind="ExternalInput")
    idxA1_d = nc.dram_tensor("idxA1", [128, LA1 // 16], I16, kind="ExternalInput")
    idxB1_d = nc.dram_tensor("idxB1", [128, LB1 // 16], I16, kind="ExternalInput")
    lblA1_d = nc.dram_tensor("lblA1", [128, GA1], BF, kind="ExternalInput")
    lblB1_d = nc.dram_tensor("lblB1", [128, GB1], BF, kind="ExternalInput")
    icnA1_d = nc.dram_tensor("icnA1", [128, GA1], BF, kind="ExternalInput")
    icnB1_d = nc.dram_tensor("icnB1", [128, GB1], BF, kind="ExternalInput")
    Wm_d = nc.dram_tensor("Wm", [128, C], BF, kind="ExternalInput")
    bm_d = nc.dram_tensor("bm", [C, 1], F32, kind="ExternalInput")
    W0_d = nc.dram_tensor("W0", [128, HID], BF, kind="ExternalInput")
    b0_d = nc.dram_tensor("b0", [HID, 1], F32, kind="ExternalInput")
    W1_d = nc.dram_tensor("W1", [HID, C], BF, kind="ExternalInput")
    b1_d = nc.dram_tensor("b1", [C, 1], F32, kind="ExternalInput")
    p0_d = nc.dram_tensor("p0", [128, NET], F32, kind="ExternalInput")
    p1_d = nc.dram_tensor("p1", [128, NET], F32, kind="ExternalInput")
    iota3_d = nc.dram_tensor("iota3", [128, 128 * CH], BF, kind="ExternalInput")
    ident_d = nc.dram_tensor("ident", [128, 128], BF, kind="ExternalInput")
    outT_d = nc.dram_tensor("outT", [C, ND], F32, kind="ExternalOutput")
    simT_d = nc.dram_tensor("simT", [C, ND], F32, kind="ExternalOutput")

    # AllGather chunk boundaries (after these block indices)
    ag_after = set()
    for c in range(1, cfg.n_ag + 1):
        ag_after.add(min(NB - 1, (c * NB) // cfg.n_ag - 1))

    with tile.TileContext(nc) as tc:
        with (
            tc.tile_pool(name="const", bufs=1) as cp,
            tc.tile_pool(name="big", bufs=1) as bigp,
            tc.tile_pool(name="ga", bufs=3) as gap,
            tc.tile_pool(name="gb", bufs=2) as gbp,
            tc.tile_pool(name="sgen", bufs=2) as sp,
            tc.tile_pool(name="work", bufs=2) as wp,
            tc.tile_pool(name="pse", bufs=3, space="PSUM") as pp,
            tc.tile_pool(name="po", bufs=3, space="PSUM") as pop,
            tc.tile_pool(name="dram", bufs=1, space="DRAM") as dp,
        ):
            # ---- resident constants ----
            idxA = cp.tile([128, LA // 16], I16)
            idxB = cp.tile([128, LB // 16], I16)
            idxA1 = cp.tile([128, LA1 // 16], I16)
            idxB1 = cp.tile([128, LB1 // 16], I16)
            nc.sync.dma_start(out=idxA[:, :], in_=idxA_d[:, :])
            nc.sync.dma_start(out=idxB[:, :], in_=idxB_d[:, :])
            nc.sync.dma_start(out=idxA1[:, :], in_=idxA1_d[:, :])
            nc.sync.dma_start(out=idxB1[:, :], in_=idxB1_d[:, :])
            lblA = cp.tile([128, GA], BF)
            lblB = cp.tile([128, GB], BF)
            icnA = cp.tile([128, GA], BF)
            icnB = cp.tile([128, GB], BF)
            lblA1 = cp.tile([128, GA1], BF)
            lblB1 = cp.tile([128, GB1], BF)
            icnA1 = cp.tile([128, GA1], BF)
            icnB1 = cp.tile([128, GB1], BF)
            for t_, d_ in [(lblA, lblA_d), (lblB, lblB_d),
                           (icnA, icnA_d), (icnB, icnB_d),
                           (lblA1, lblA1_d), (lblB1, lblB1_d),
                           (icnA1, icnA1_d), (icnB1, icnB1_d)]:
                nc.sync.dma_start(out=t_[:], in_=d_[:, :])
            x_ownT = cp.tile([128, NB * 128], BF)
            nc.sync.dma_start(out=x_ownT[:], in_=x_ownT_d[:, :])
            Wm_s = cp.tile([128, C], BF)
            bm_s = cp.tile([C, 1], F32)
            W0_s = cp.tile([128, HID], BF)
            b0_s = cp.tile([HID, 1], F32)
            W1_s = cp.tile([HID, C], BF)
            b1_s = cp.tile([C, 1], F32)
            p0_s = cp.tile([128, NET], F32)
            p1_s = cp.tile([128, NET], F32)
            iota3 = cp.tile([128, 128, CH], BF)
            ident_s = cp.tile([128, 128], BF)
            for t_, d_ in [
                (Wm_s, Wm_d), (bm_s, bm_d), (W0_s, W0_d), (b0_s, b0_d),
                (W1_s, W1_d), (b1_s, b1_d), (p0_s, p0_d), (p1_s, p1_d),
                (iota3, iota3_d.rearrange("p (j g) -> p j g", j=128)),
                (ident_s, ident_d),
            ]:
                nc.sync.dma_start(out=t_[:], in_=d_[:, :])

            hb = bigp.tile([128, NB, 128], BF)   # [h(64) | 0] feature-major
            nc.vector.memset(hb[:, :, :], 0)

            h_loc = dp.tile([ND, 128], BF)
            h_ag = dp.tile([N, 128], BF)

            qctr = [0]

            def gather_batched(gtile, src, idx_tile, gc0, nchunks, tag):
                c = 0
                while c < nchunks:
                    cc = min(8, nchunks - c)
                    n = cc * 128
                    q = qctr[0] % 4
                    qctr[0] += 1
                    nc.gpsimd.dma_gather(
                        gtile[:, c : c + cc, :], src,
                        idx_tile[:, (gc0 + c) * 8 : (gc0 + c + cc) * 8],
                        n, n, 128, queue_num=q,
                    )
                    c += cc

            def make_S(S, lbl_t, icn_t, gc0, CX):
                nc.vector.tensor_tensor(
                    S[:, :, :CX], iota3[:, :, :CX],
                    lbl_t[:, gc0 : gc0 + CX].unsqueeze(1).broadcast_to(
                        [128, 128, CX]),
                    ISEQ,
                )
                nc.vector.tensor_tensor(
                    S[:, :, :CX], S[:, :, :CX],
                    icn_t[:, gc0 : gc0 + CX].unsqueeze(1).broadcast_to(
                        [128, 128, CX]),
                    MULT,
                )

            def layer(lnum, src_lo, src_hi, p_s, ag_row0, meta):
                """One CARE layer; returns nothing (writes outputs)."""
                ixA, ixB, lbA, lbB, icA, icB, CAx, CBx = meta
                for b in range(NB):
                    ts = []
                    for i in range(NET):
                        gcA = (i * NB + b) * CAx
                        gcB = (i * NB + b) * CBx
                        gA = gap.tile([128, CAx, 128], BF, tag=f"gA{lnum}")
                        gather_batched(gA, src_lo, ixA, gcA, CAx, "A")
                        # B stream: calls span 2 blocks (CBx<=4 each)
                        if b % 2 == 0:
                            gBt = gbp.tile([128, 2 * CBx, 128], BF,
                                           tag=f"gB{lnum}{i}")
                            nch = min(2 * CBx, (NB - b) * CBx)
                            gather_batched(gBt, src_hi, ixB, gcB, nch, "B")
                            self_gB = gBt
                        else:
                            self_gB = None
                        SA = sp.tile([128, 128, CAx], BF, tag=f"SA{lnum}")
                        SB = sp.tile([128, 128, CBx], BF, tag=f"SB{lnum}")
                        make_S(SA, lbA, icA, gcA, CAx)
                        make_S(SB, lbB, icB, gcB, CBx)
                        ps = pp.tile([128, 128], F32, tag="ps")
                        for c in range(CAx):
                            nc.tensor.matmul(
                                ps[:, :], gA[:, c, :], SA[:, :, c],
                                start=(c == 0), stop=False,
                            )
                        gB_use = self_gB if self_gB is not None else gB_prev[i]
                        boff = (b % 2) * CBx
                        for c in range(CBx):
                            nc.tensor.matmul(
                                ps[:, :], gB_use[:, boff + c, :],
                                SB[:, :, c],
                                start=False, stop=(c == CBx - 1),
                            )
                        if b % 2 == 0:
                            gB_prev[i] = self_gB
                        # tanh(mean) from PSUM
                        t = wp.tile([128, 128], F32, tag=f"t{i}")
                        nc.scalar.activation(t[:], ps[:], TANH)
                        ts.append(t)
                    # combine: acc = sum p_i * t_i + residual
                    acc = wp.tile([128, 128], F32, tag="acc")
                    nc.scalar.mul(acc[:], ts[0][:], p_s[:, 0:1])
                    for i in range(1, NET):
                        tmp = wp.tile([128, 128], F32, tag="tmp")
                        nc.scalar.mul(tmp[:], ts[i][:], p_s[:, i : i + 1])
                        nc.vector.tensor_tensor(acc[:], acc[:], tmp[:], ADD)
                    if lnum == 0:
                        nc.vector.tensor_tensor(
                            acc[:], acc[:],
                            x_ownT[:, b * 128 : (b + 1) * 128], ADD)
                        nc.scalar.activation(acc[:], acc[:], TANH)
                        h0b = wp.tile([128, 128], BF, tag="h0b")
                        nc.vector.tensor_copy(h0b[:], acc[:])
                        # hnat = h0b @ W0 + b0  -> [64, 128]
                        po_t = pop.tile([128, 128], F32, tag="pp")
                        po = po_t[0:HID, :]
                        nc.tensor.matmul(po[:], W0_s[:], h0b[:])
                        nc.vector.tensor_scalar(
                            hb[0:HID, b, :], po[:], b0_s[:, 0:1], None, ADD)
                        # sim = tanh(x @ Wm + bm) -> [2, 128]
                        psim_t = pop.tile([128, 128], F32, tag="pp")
                        psim = psim_t[0:C, :]
                        nc.tensor.matmul(
                            psim[:], Wm_s[:],
                            x_ownT[:, b * 128 : (b + 1) * 128])
                        n = cfg.bs(b)
                        so = wp.tile([C, 128], F32, tag="so")
                        nc.scalar.activation(
                            so[:], psim[:], TANH, bias=bm_s[:, 0:1])
                        nc.sync.dma_start(
                            out=simT_d[:, b * 128 : b * 128 + n],
                            in_=so[:, 0:n])
                        # transpose hb block -> h_loc rows (via matmul w/ ident)
                        pt = pop.tile([128, 128], F32, tag="pp")
                        nc.tensor.matmul(pt[:], hb[:, b, :], ident_s[:])
                        hrow = wp.tile([128, 128], BF, tag="hrow")
                        nc.vector.tensor_copy(hrow[:], pt[:])
                        nc.sync.dma_start(
                            out=h_loc[b * 128 : b * 128 + n, :],
                            in_=hrow[0:n, :])
                        if b in ag_after:
                            r0 = ag_row0[0]
                            r1 = min(ND, (b + 1) * 128)
                            if r1 > r0:
                                base = cfg.n_cores * r0
                                outv = h_ag[base : base + cfg.n_cores
                                            * (r1 - r0), :].rearrange(
                                    "(k r) d -> k r d", k=cfg.n_cores)
                                nc.gpsimd.collective_compute(
                                    "AllGather",
                                    mybir.AluOpType.bypass,
                                    replica_groups=[list(range(cfg.n_cores))],
                                    ins=[h_loc[r0:r1, :].opt()],
                                    outs=[outv.opt()],
                                )
                            ag_row0[0] = r1
                    else:
                        nc.vector.tensor_tensor(
                            acc[0:HID, :], acc[0:HID, :], hb[0:HID, b, :], ADD)
                        nc.scalar.activation(
                            acc[0:HID, :], acc[0:HID, :], TANH)
                        h2b = wp.tile([HID, 128], BF, tag="h2b")
                        nc.vector.tensor_copy(h2b[:], acc[0:HID, :])
                        po2_t = pop.tile([128, 128], F32, tag="pp")
                        po2 = po2_t[0:C, :]
                        nc.tensor.matmul(po2[:], W1_s[:], h2b[:])
                        n = cfg.bs(b)
                        oo = wp.tile([C, 128], F32, tag="oo")
                        nc.vector.tensor_scalar(
                            oo[:], po2[:], b1_s[:, 0:1], None, ADD)
                        nc.sync.dma_start(
                            out=outT_d[:, b * 128 : b * 128 + n],
                            in_=oo[:, 0:n])

            gB_prev = [None] * NET
            layer(0, x_rows_d[0:SPLIT, :], x_rows_d[SPLIT:N, :], p0_s, [0],
                  (idxA, idxB, lblA, lblB, icnA, icnB, CA, CB))
            gB_prev = [None] * NET
            layer(1, h_ag[0:SPLIT, :], h_ag[SPLIT:N, :], p1_s, [ND],
                  (idxA1, idxB1, lblA1, lblB1, icnA1, icnB1, CA1, CB1))

    nc.compile()
    return nc


_CACHE = {}


def _get_nc(cfg, CAB):
    key = (cfg.N, cfg.E, cfg.n_cores, CAB)
    if key not in _CACHE:
        _CACHE[key] = build_nc(cfg, CAB)
    return _CACHE[key]


def kernel(**inputs):
    cfg = Cfg()
    in_maps, CAB = host_prep(cfg, inputs)
    nc = _get_nc(cfg, CAB)
    res = run_bass_kernel_spmd(nc, in_maps, core_ids=list(range(cfg.n_cores)))
    out = np.concatenate(
        [r["outT"] for r in res.results], axis=1
    ).T.astype(np.float32)
    sim = np.concatenate(
        [r["simT"] for r in res.results], axis=1
    ).T.astype(np.float32)
    return (np.ascontiguousarray(out), np.ascontiguousarray(sim))


# revision 7
# speedup vs baseline: 1.3521x; 1.2675x over previous
"""CARE-GNN Trainium2 kernel (nn_CAREGNN_62199716381202), v2.

Strategy (graph/data parallel, 8 NeuronCores):
- Shard destination nodes across the 8 cores (6250 dsts each); each core owns
  the edges incident (by dst) to its shard, sorted by dst, split into
  low-src / high-src streams (int16 gather-index limit at 32768).
- Per 128-dst block and etype, edges are gathered (dma_gather, bf16 rows,
  256B descriptors) into edge-major chunks G [128 slots, 128 feat].
- Segment-MEAN via one flipped one-hot matmul per chunk:
      psum[feat, dst] += G[slot, feat]^T @ S[slot, dst]
  where S = (iota == lbl) * (1/deg)  -- the mean scale is baked into S, so
  PSUM holds the per-etype mean directly, already in transposed [feat, dst]
  layout.  The transposed layout feeds tanh / p-weighted sum / residual /
  Linear (W @ h) with no per-block PE transposes.
- bf16 everywhere data-sized (inputs quantized ~0.4%, fine for rel<2e-2).
- Both layers share identical edge streams (same graph): one set of
  idx/label/icnt metadata, two gather sources (X rows, AllGathered H rows).
- AllGather of layer-0 H rows (bf16, 128-padded) is chunked to overlap with
  layer-0 compute.
"""

import sys

if "/opt/trn_rl_repo" not in sys.path:
    sys.path.insert(0, "/opt/trn_rl_repo")

import numpy as np
import ml_dtypes

BF16 = ml_dtypes.bfloat16

import concourse.bass as bass
import concourse.bacc as bacc
import concourse.mybir as mybir
import concourse.tile as tile
from concourse.bass_utils import run_bass_kernel_spmd

F32 = mybir.dt.float32
BF = mybir.dt.bfloat16
I16 = mybir.dt.int16
ADD = mybir.AluOpType.add
MULT = mybir.AluOpType.mult
ISEQ = mybir.AluOpType.is_equal
TANH = mybir.ActivationFunctionType.Tanh


class Cfg:
    def __init__(self, N=50000, E=500000, n_cores=8, split=32768, n_ag=4):
        self.N = N
        self.E = E
        self.D = 128
        self.HID = 64
        self.C = 2
        self.NET = 3
        self.n_cores = n_cores
        self.split = split
        self.n_ag = n_ag
        assert N % n_cores == 0
        self.ND = N // n_cores
        self.NB = (self.ND + 127) // 128

    def bs(self, b):
        return min(128, self.ND - b * 128)


def _wrap16(flat):
    w = np.ascontiguousarray(flat.reshape(-1, 16).T).astype(np.int16)
    return np.tile(w, (8, 1))


def host_prep(cfg, inputs):
    """Build per-core input maps. Returns (in_maps, CA, CB)."""
    feat = np.asarray(inputs["feat"], np.float32)
    srcs = [np.asarray(inputs[f"src{i}"]) for i in range(cfg.NET)]
    dsts = [np.asarray(inputs[f"dst{i}"]) for i in range(cfg.NET)]

    x_rows = feat.astype(BF16)                      # [N, 128] gather source
    xT = np.ascontiguousarray(feat.T)               # [128, N] f32

    # pass 1: per (core, etype, block, half) counts -> CA, CB
    percore = []
    CA = CB = 1
    for k in range(cfg.n_cores):
        rows = []
        for i in range(cfg.NET):
            sel = (dsts[i] >= k * cfg.ND) & (dsts[i] < (k + 1) * cfg.ND)
            dl = (dsts[i][sel] - k * cfg.ND).astype(np.int64)
            s = srcs[i][sel].astype(np.int64)
            o = np.argsort(dl, kind="stable")
            dl, s = dl[o], s[o]
            deg = np.bincount(dl, minlength=cfg.ND)
            b = dl >> 7
            half = (s >= cfg.split).astype(np.int64)
            for b_ in range(cfg.NB):
                mb = b == b_
                nA = int((mb & (half == 0)).sum())
                nB = int((mb & (half == 1)).sum())
                CA = max(CA, -(-nA // 128))
                CB = max(CB, -(-nB // 128))
            rows.append((dl, s, b, half, deg))
        percore.append(rows)

    # layer-1 table row remap (chunk-major AllGather layout)
    bounds = [0]
    for c in range(1, cfg.n_ag + 1):
        bb = min(cfg.NB - 1, (c * cfg.NB) // cfg.n_ag - 1)
        bounds.append(min(cfg.ND, (bb + 1) * 128))
    bounds = sorted(set(bounds))
    remap = np.zeros(cfg.N, np.int64)
    nodes = np.arange(cfg.N)
    kk, rr = nodes // cfg.ND, nodes % cfg.ND
    cum = 0
    for ci in range(len(bounds) - 1):
        r0, r1 = bounds[ci], bounds[ci + 1]
        m = (rr >= r0) & (rr < r1)
        remap[m] = cum + kk[m] * (r1 - r0) + (rr[m] - r0)
        cum += cfg.n_cores * (r1 - r0)

    # layer-1 chunk maxima (remapped halves)
    CA1 = CB1 = 1
    for k in range(cfg.n_cores):
        for i in range(cfg.NET):
            dl, s, b, half, deg = percore[k][i]
            h1 = (remap[s] >= cfg.split).astype(np.int64)
            for b_ in range(cfg.NB):
                mb = b == b_
                nA = int((mb & (h1 == 0)).sum())
                nB = int((mb & (h1 == 1)).sum())
                CA1 = max(CA1, -(-nA // 128))
                CB1 = max(CB1, -(-nB // 128))

    LA = cfg.NET * cfg.NB * CA * 128
    LB = cfg.NET * cfg.NB * CB * 128
    GA = cfg.NET * cfg.NB * CA   # lo chunks
    GB = cfg.NET * cfg.NB * CB   # hi chunks
    LA1 = cfg.NET * cfg.NB * CA1 * 128
    LB1 = cfg.NET * cfg.NB * CB1 * 128
    GA1 = cfg.NET * cfg.NB * CA1
    GB1 = cfg.NET * cfg.NB * CB1

    Wm = np.asarray(inputs["Wm"], np.float32).astype(BF16)   # [128, 2]
    bm = np.asarray(inputs["bm"], np.float32).reshape(cfg.C, 1)
    W0 = np.asarray(inputs["W0"], np.float32).astype(BF16)   # [128, 64]
    b0 = np.asarray(inputs["b0"], np.float32).reshape(cfg.HID, 1)
    W1 = np.asarray(inputs["W1"], np.float32).astype(BF16)   # [64, 2]
    b1 = np.asarray(inputs["b1"], np.float32).reshape(cfg.C, 1)
    p0 = np.tile(np.asarray(inputs["p0"], np.float32), (128, 1))
    p1 = np.tile(np.asarray(inputs["p1"], np.float32), (128, 1))
    CH = max(CA, CB, CA1, CB1)
    iota3 = np.ascontiguousarray(
        np.broadcast_to(
            np.arange(128, dtype=np.float32).astype(BF16)[None, :, None],
            (128, 128, CH),
        ).reshape(128, 128 * CH)
    )
    ident = np.eye(128, dtype=np.float32).astype(BF16)

    def build_meta(k, CXA, CXB, use_remap):
        LAx, LBx = cfg.NET * cfg.NB * CXA * 128, cfg.NET * cfg.NB * CXB * 128
        GAx, GBx = cfg.NET * cfg.NB * CXA, cfg.NET * cfg.NB * CXB
        idxA = np.zeros(LAx, np.int64)
        idxB = np.zeros(LBx, np.int64)
        lblA = np.full((128, GAx), -7.0, np.float32)
        lblB = np.full((128, GBx), -7.0, np.float32)
        for i in range(cfg.NET):
            dl, s, b, half, deg = percore[k][i]
            sv = remap[s] if use_remap else s
            hv = (sv >= cfg.split).astype(np.int64)
            ic = 1.0 / np.maximum(deg, 1.0)
            for half_, (idx_, lbl_, CX) in (
                (0, (idxA, lblA, CXA)),
                (1, (idxB, lblB, CXB)),
            ):
                m = hv == half_
                dlh, sh = dl[m], sv[m]
                bh = dlh >> 7
                cnt = np.bincount(bh, minlength=cfg.NB)
                start = np.zeros(cfg.NB + 1, np.int64)
                np.cumsum(cnt, out=start[1:])
                j = np.arange(len(dlh)) - start[bh]
                pos = ((i * cfg.NB + bh) * CX + (j >> 7)) * 128 + (j & 127)
                idx_[pos] = sh - (cfg.split if half_ else 0)
                ch = (i * cfg.NB + bh) * CX + (j >> 7)
                lbl_[j & 127, ch] = dlh - bh * 128
        return idxA, idxB, lblA, lblB

    in_maps = []
    for k in range(cfg.n_cores):
        idxA, idxB, lblA, lblB = build_meta(k, CA, CB, False)
        idxA1, idxB1, lblA1, lblB1 = build_meta(k, CA1, CB1, True)
        icr = np.zeros((128, cfg.NET * cfg.NB * 128), np.float32)
        for i in range(cfg.NET):
            deg = percore[k][i][4]
            ic = 1.0 / np.maximum(deg, 1.0)
            icr[:, i * cfg.NB * 128 : i * cfg.NB * 128 + cfg.ND] = ic[None, :]
        xo = np.zeros((128, cfg.NB * 128), BF16)
        xo[:, : cfg.ND] = xT[:, k * cfg.ND : (k + 1) * cfg.ND].astype(BF16)
        in_maps.append(
            {
                "x_rows": x_rows,
                "x_ownT": xo,
                "idxA": _wrap16(idxA),
                "idxB": _wrap16(idxB),
                "lblA": lblA.astype(BF16),
                "lblB": lblB.astype(BF16),
                "icnt_rep": icr.astype(BF16),
                "idxA1": _wrap16(idxA1),
                "idxB1": _wrap16(idxB1),
                "lblA1": lblA1.astype(BF16),
                "lblB1": lblB1.astype(BF16),
                "Wm": Wm, "bm": bm, "W0": W0, "b0": b0, "W1": W1, "b1": b1,
                "p0": p0, "p1": p1, "iota3": iota3, "ident": ident,
            }
        )
    return in_maps, (CA, CB, CA1, CB1)


def build_nc(cfg, CAB, debug=False):
    CA, CB, CA1, CB1 = CAB
    N, ND, NB, NET, HID, C = cfg.N, cfg.ND, cfg.NB, cfg.NET, cfg.HID, cfg.C
    SPLIT = cfg.split
    LA = NET * NB * CA * 128
    LB = NET * NB * CB * 128
    GA = NET * NB * CA
    GB = NET * NB * CB
    LA1 = NET * NB * CA1 * 128
    LB1 = NET * NB * CB1 * 128
    GA1 = NET * NB * CA1
    GB1 = NET * NB * CB1
    CH = max(CA, CB, CA1, CB1)

    nc = bacc.Bacc(trn_type="TRN2", num_devices=cfg.n_cores,
                   num_swdge_queues=4)

    x_rows_d = nc.dram_tensor("x_rows", [N, 128], BF, kind="ExternalInput")
    x_ownT_d = nc.dram_tensor("x_ownT", [128, NB * 128], BF, kind="ExternalInput")
    idxA_d = nc.dram_tensor("idxA", [128, LA // 16], I16, kind="ExternalInput")
    idxB_d = nc.dram_tensor("idxB", [128, LB // 16], I16, kind="ExternalInput")
    lblA_d = nc.dram_tensor("lblA", [128, GA], BF, kind="ExternalInput")
    lblB_d = nc.dram_tensor("lblB", [128, GB], BF, kind="ExternalInput")
    icnt_d = nc.dram_tensor("icnt_rep", [128, NET * NB * 128], BF,
                            kind="ExternalInput")
    idxA1_d = nc.dram_tensor("idxA1", [128, LA1 // 16], I16, kind="ExternalInput")
    idxB1_d = nc.dram_tensor("idxB1", [128, LB1 // 16], I16, kind="ExternalInput")
    lblA1_d = nc.dram_tensor("lblA1", [128, GA1], BF, kind="ExternalInput")
    lblB1_d = nc.dram_tensor("lblB1", [128, GB1], BF, kind="ExternalInput")
    Wm_d = nc.dram_tensor("Wm", [128, C], BF, kind="ExternalInput")
    bm_d = nc.dram_tensor("bm", [C, 1], F32, kind="ExternalInput")
    W0_d = nc.dram_tensor("W0", [128, HID], BF, kind="ExternalInput")
    b0_d = nc.dram_tensor("b0", [HID, 1], F32, kind="ExternalInput")
    W1_d = nc.dram_tensor("W1", [HID, C], BF, kind="ExternalInput")
    b1_d = nc.dram_tensor("b1", [C, 1], F32, kind="ExternalInput")
    p0_d = nc.dram_tensor("p0", [128, NET], F32, kind="ExternalInput")
    p1_d = nc.dram_tensor("p1", [128, NET], F32, kind="ExternalInput")
    iota3_d = nc.dram_tensor("iota3", [128, 128 * CH], BF, kind="ExternalInput")
    ident_d = nc.dram_tensor("ident", [128, 128], BF, kind="ExternalInput")
    outT_d = nc.dram_tensor("outT", [C, ND], F32, kind="ExternalOutput")
    simT_d = nc.dram_tensor("simT", [C, ND], F32, kind="ExternalOutput")

    # AllGather chunk boundaries (after these block indices)
    ag_after = set()
    for c in range(1, cfg.n_ag + 1):
        ag_after.add(min(NB - 1, (c * NB) // cfg.n_ag - 1))

    with tile.TileContext(nc) as tc:
        with (
            tc.tile_pool(name="const", bufs=1) as cp,
            tc.tile_pool(name="big", bufs=1) as bigp,
            tc.tile_pool(name="ga", bufs=6) as gap,
            tc.tile_pool(name="gb", bufs=2) as gbp,
            tc.tile_pool(name="sgen", bufs=2) as sp,
            tc.tile_pool(name="work", bufs=2) as wp,
            tc.tile_pool(name="pse", bufs=3, space="PSUM") as pp,
            tc.tile_pool(name="po", bufs=3, space="PSUM") as pop,
            tc.tile_pool(name="dram", bufs=1, space="DRAM") as dp,
        ):
            # ---- resident constants ----
            LAm, LBm = max(LA, LA1), max(LB, LB1)
            lblA = cp.tile([128, GA], BF)
            lblB = cp.tile([128, GB], BF)
            lblA1 = cp.tile([128, GA1], BF)
            lblB1 = cp.tile([128, GB1], BF)
            icnt_rep = cp.tile([128, NET * NB * 128], BF)
            for t_, d_ in [(lblA, lblA_d), (lblB, lblB_d),
                           (lblA1, lblA1_d), (lblB1, lblB1_d),
                           (icnt_rep, icnt_d)]:
                nc.sync.dma_start(out=t_[:], in_=d_[:, :])
            x_ownT = cp.tile([128, NB * 128], BF)
            nc.sync.dma_start(out=x_ownT[:], in_=x_ownT_d[:, :])
            Wm_s = cp.tile([128, C], BF)
            bm_s = cp.tile([C, 1], F32)
            W0_s = cp.tile([128, HID], BF)
            b0_s = cp.tile([HID, 1], F32)
            W1_s = cp.tile([HID, C], BF)
            b1_s = cp.tile([C, 1], F32)
            p0_s = cp.tile([128, NET], F32)
            p1_s = cp.tile([128, NET], F32)
            iota3 = cp.tile([128, 128, CH], BF)
            ident_s = cp.tile([128, 128], BF)
            for t_, d_ in [
                (Wm_s, Wm_d), (bm_s, bm_d), (W0_s, W0_d), (b0_s, b0_d),
                (W1_s, W1_d), (b1_s, b1_d), (p0_s, p0_d), (p1_s, p1_d),
                (iota3, iota3_d.rearrange("p (j g) -> p j g", j=128)),
                (ident_s, ident_d),
            ]:
                nc.sync.dma_start(out=t_[:], in_=d_[:, :])

            hb = bigp.tile([128, NB, 128], BF)   # [h(64) | 0] feature-major
            nc.vector.memset(hb[:, :, :], 0)

            h_loc = dp.tile([ND, 128], BF)
            h_ag = dp.tile([N, 128], BF)

            qctr = [0]

            def gather_batched(gtile, src, idx_tile, gc0, nchunks, tag):
                c = 0
                while c < nchunks:
                    cc = min(8, nchunks - c)
                    n = cc * 128
                    q = qctr[0] % 4
                    qctr[0] += 1
                    nc.gpsimd.dma_gather(
                        gtile[:, c : c + cc, :], src,
                        idx_tile[:, (gc0 + c) * 8 : (gc0 + c + cc) * 8],
                        n, n, 128, queue_num=q,
                    )
                    c += cc

            def make_S(S, lbl_t, gc0, CX):
                nc.vector.tensor_tensor(
                    S[:, :, :CX], iota3[:, :, :CX],
                    lbl_t[:, gc0 : gc0 + CX].unsqueeze(1).broadcast_to(
                        [128, 128, CX]),
                    ISEQ,
                )

            def layer(lnum, src_lo, src_hi, p_s, ag_row0, meta):
                """One CARE layer; returns nothing (writes outputs)."""
                ixA_d, ixB_d, lbA, lbB, CAx, CBx = meta
                LAx = NET * NB * CAx * 128
                LBx = NET * NB * CBx * 128
                ixA = wp.tile([128, LAm // 16], I16, tag="ixA")
                ixB = wp.tile([128, LBm // 16], I16, tag="ixB")
                nc.sync.dma_start(out=ixA[:, 0 : LAx // 16], in_=ixA_d[:, :])
                nc.sync.dma_start(out=ixB[:, 0 : LBx // 16], in_=ixB_d[:, :])
                for b in range(NB):
                    ts = []
                    for i in range(NET):
                        gcA = (i * NB + b) * CAx
                        gcB = (i * NB + b) * CBx
                        gA = gap.tile([128, CAx, 128], BF, tag=f"gA{lnum}")
                        gather_batched(gA, src_lo, ixA, gcA, CAx, "A")
                        # B stream: calls span 2 blocks (CBx<=4 each)
                        if b % 2 == 0:
                            gBt = gbp.tile([128, 2 * CBx, 128], BF,
                                           tag=f"gB{lnum}{i}")
                            nch = min(2 * CBx, (NB - b) * CBx)
                            gather_batched(gBt, src_hi, ixB, gcB, nch, "B")
                            self_gB = gBt
                        else:
                            self_gB = None
                        SA = sp.tile([128, 128, CAx], BF, tag=f"SA{lnum}")
                        SB = sp.tile([128, 128, CBx], BF, tag=f"SB{lnum}")
                        make_S(SA, lbA, gcA, CAx)
                        make_S(SB, lbB, gcB, CBx)
                        ps = pp.tile([128, 128], F32, tag="ps")
                        for c in range(CAx):
                            nc.tensor.matmul(
                                ps[:, :], gA[:, c, :], SA[:, :, c],
                                start=(c == 0), stop=False,
                            )
                        gB_use = self_gB if self_gB is not None else gB_prev[i]
                        boff = (b % 2) * CBx
                        for c in range(CBx):
                            nc.tensor.matmul(
                                ps[:, :], gB_use[:, boff + c, :],
                                SB[:, :, c],
                                start=False, stop=(c == CBx - 1),
                            )
                        if b % 2 == 0:
                            gB_prev[i] = self_gB
                        # mean = sum * (1/deg); tanh on scalar engine
                        t = wp.tile([128, 128], F32, tag=f"t{i}")
                        nc.vector.tensor_tensor(
                            t[:], ps[:],
                            icnt_rep[:, (i * NB + b) * 128
                                     : (i * NB + b + 1) * 128], MULT)
                        nc.scalar.activation(t[:], t[:], TANH)
                        ts.append(t)
                    # combine: acc = sum p_i * t_i + residual
                    acc = wp.tile([128, 128], F32, tag="acc")
                    nc.scalar.mul(acc[:], ts[0][:], p_s[:, 0:1])
                    for i in range(1, NET):
                        tmp = wp.tile([128, 128], F32, tag="tmp")
                        nc.scalar.mul(tmp[:], ts[i][:], p_s[:, i : i + 1])
                        nc.vector.tensor_tensor(acc[:], acc[:], tmp[:], ADD)
                    if lnum == 0:
                        nc.vector.tensor_tensor(
                            acc[:], acc[:],
                            x_ownT[:, b * 128 : (b + 1) * 128], ADD)
                        h0b = wp.tile([128, 128], BF, tag="h0b")
                        nc.scalar.activation(h0b[:], acc[:], TANH)
                        # hnat = h0b @ W0 + b0  -> [64, 128]
                        po_t = pop.tile([128, 128], F32, tag="pp")
                        po = po_t[0:HID, :]
                        nc.tensor.matmul(po[:], W0_s[:], h0b[:])
                        nc.vector.tensor_scalar(
                            hb[0:HID, b, :], po[:], b0_s[:, 0:1], None, ADD)
                        # sim = tanh(x @ Wm + bm) -> [2, 128]
                        psim_t = pop.tile([128, 128], F32, tag="pp")
                        psim = psim_t[0:C, :]
                        nc.tensor.matmul(
                            psim[:], Wm_s[:],
                            x_ownT[:, b * 128 : (b + 1) * 128])
                        n = cfg.bs(b)
                        so = wp.tile([C, 128], F32, tag="so")
                        nc.scalar.activation(
                            so[:], psim[:], TANH, bias=bm_s[:, 0:1])
                        nc.sync.dma_start(
                            out=simT_d[:, b * 128 : b * 128 + n],
                            in_=so[:, 0:n])
                        # transpose hb block -> h_loc rows (via matmul w/ ident)
                        pt = pop.tile([128, 128], F32, tag="pp")
                        nc.tensor.matmul(pt[:], hb[:, b, :], ident_s[:])
                        hrow = wp.tile([128, 128], BF, tag="hrow")
                        nc.scalar.copy(hrow[:], pt[:])
                        nc.sync.dma_start(
                            out=h_loc[b * 128 : b * 128 + n, :],
                            in_=hrow[0:n, :])
                        if b in ag_after:
                            r0 = ag_row0[0]
                            r1 = min(ND, (b + 1) * 128)
                            if r1 > r0:
                                base = cfg.n_cores * r0
                                outv = h_ag[base : base + cfg.n_cores
                                            * (r1 - r0), :].rearrange(
                                    "(k r) d -> k r d", k=cfg.n_cores)
                                nc.gpsimd.collective_compute(
                                    "AllGather",
                                    mybir.AluOpType.bypass,
                                    replica_groups=[list(range(cfg.n_cores))],
                                    ins=[h_loc[r0:r1, :].opt()],
                                    outs=[outv.opt()],
                                )
                            ag_row0[0] = r1
                    else:
                        nc.vector.tensor_tensor(
                            acc[0:HID, :], acc[0:HID, :], hb[0:HID, b, :], ADD)
                        h2b = wp.tile([HID, 128], BF, tag="h2b")
                        nc.scalar.activation(h2b[:], acc[0:HID, :], TANH)
                        po2_t = pop.tile([128, 128], F32, tag="pp")
                        po2 = po2_t[0:C, :]
                        nc.tensor.matmul(po2[:], W1_s[:], h2b[:])
                        n = cfg.bs(b)
                        oo = wp.tile([C, 128], F32, tag="oo")
                        nc.vector.tensor_scalar(
                            oo[:], po2[:], b1_s[:, 0:1], None, ADD)
                        nc.sync.dma_start(
                            out=outT_d[:, b * 128 : b * 128 + n],
                            in_=oo[:, 0:n])

            gB_prev = [None] * NET
            layer(0, x_rows_d[0:SPLIT, :], x_rows_d[SPLIT:N, :], p0_s, [0],
                  (idxA_d, idxB_d, lblA, lblB, CA, CB))
            gB_prev = [None] * NET
            layer(1, h_ag[0:SPLIT, :], h_ag[SPLIT:N, :], p1_s, [ND],
                  (idxA1_d, idxB1_d, lblA1, lblB1, CA1, CB1))

    nc.compile()
    return nc


_CACHE = {}


def _get_nc(cfg, CAB):
    key = (cfg.N, cfg.E, cfg.n_cores, CAB)
    if key not in _CACHE:
        _CACHE[key] = build_nc(cfg, CAB)
    return _CACHE[key]


def kernel(**inputs):
    cfg = Cfg()
    in_maps, CAB = host_prep(cfg, inputs)
    nc = _get_nc(cfg, CAB)
    res = run_bass_kernel_spmd(nc, in_maps, core_ids=list(range(cfg.n_cores)))
    out = np.concatenate(
        [r["outT"] for r in res.results], axis=1
    ).T.astype(np.float32)
    sim = np.concatenate(
        [r["simT"] for r in res.results], axis=1
    ).T.astype(np.float32)
    return (np.ascontiguousarray(out), np.ascontiguousarray(sim))
